# revision 27
# baseline (speedup 1.0000x reference)
"""Trainium2 Bass kernel for a GNN message-passing layer (BoundaryConvLayer).

Computation (reference, per node i over D=128 channels):
    rate  = softplus(x @ W_rate) + EPS
    gamma = x @ W_rob + b_rob
    h     = x @ W_fc + b_fc
    agg   = segment_sum(h[row] + h[col], row)
    y     = LayerNorm((rate*agg + gamma) / (1 + rate*deg + EPS)) * ln_gamma + ln_beta

Distribution: nodes sharded across 8 cores by contiguous row blocks; edges
partitioned by destination row so the segment sum is local to each core.

Key identity (g = x @ W_fc, cnt = in-edge count):
    agg[i] = (sum_{e:row=i} x[col_e]  +  cnt[i]*x[i]) @ W_fc + 2*cnt[i]*b_fc
The segment sum runs in INPUT space: the host stages the per-edge source
rows x[col_e] (pure indexing, no host FLOPs) grouped by destination tile,
and the PE reduces each 128-slot group with a one-hot "selection matrix"
matmul accumulated in PSUM; the self term is an extra slot group holding
the tile's own rows with sel = diag(cnt).  This removes the device-side
table gather (a software-DGE bottleneck) and the redundant full-N GEMM:
all DMA is large contiguous hardware-queue traffic.

Throughput notes:
  - Software pipelined: super-group g's elementwise/LayerNorm is emitted
    AFTER super-group g+1's matmul work, so the PE never starves at SG
    boundaries waiting for DVE to finish the previous tail.
  - PSUM banks are packed (4 agg tiles / 2 xw2 tiles per 2KB bank) so the
    PSUM->SBUF copies are few and wide; the agg phase of a completed quad
    is interleaved one tile later to hide the copy latency.
  - Elementwise runs on [128, 896] bf16 operands (DVE 2x mode where the
    access pattern allows), reductions on Pool, exp/ln chains on ACT.
  - 1/den and rsqrt go through exp/ln so one ACT table load suffices.
"""

import numpy as np
import ml_dtypes
from contextlib import ExitStack
from dataclasses import dataclass

import concourse.bass as bass
import concourse.tile as tile
from concourse import bacc, mybir
from concourse.bass_utils import run_bass_kernel_spmd

# The stock ACT-table chooser greedily picks the first set containing each
# function, which for {Exp, Ln, Copy, Square} can alternate between two sets
# and reload the table per use (~1.3us each).  Restrict it to the one set
# that contains all of them so a single load suffices.
_ACT_KEEP = "natural_log_exp_and_others"
if not getattr(bacc, "_act_tables_patched", False):
    _orig_get_tables = bacc.get_activation_tables

    def _patched_get_tables(arch):
        t = _orig_get_tables(arch)
        if _ACT_KEEP in t:
            t = {k: (v if k == _ACT_KEEP else set()) for k, v in t.items()}
        return t

    bacc.get_activation_tables = _patched_get_tables
    bacc._act_tables_patched = True

BF16 = ml_dtypes.bfloat16
EPS = 1e-4
LN_EPS = 1e-5
P = 128
D = 128


@dataclass
class Cfg:
    N: int            # total nodes
    E: int            # total edges
    NC: int           # cores
    S: int = 0        # edge slot groups per tile (set by prep)
    SG: int = 14      # tiles per super-group (pipelining granule)
    ln_trivial: bool = False

    @property
    def NLOC(self):
        return self.N // self.NC

    @property
    def T(self):
        return (self.NLOC + P - 1) // P

    @property
    def TLP(self):
        return self.T * P

    @property
    def NSG(self):
        assert self.T % self.SG == 0
        return self.T // self.SG

    @property
    def SP1(self):    # slot groups incl. the self group
        return self.S + 1


def prep(x, edge_index, degree, W_fc, b_fc, W_rate, W_rob, b_rob, ln_gamma, ln_beta,
         cfg: Cfg):
    """Host-side preprocessing: shard + stage per-edge source rows by dst tile."""
    N, NC = cfg.N, cfg.NC
    NLOC, T, TLP = cfg.NLOC, cfg.T, cfg.TLP

    x = np.asarray(x, np.float32)
    edge_index = np.asarray(edge_index, np.int64)
    degree = np.asarray(degree)
    row, col = edge_index[0], edge_index[1]

    xbf = x.astype(BF16)
    xbf_ext = np.concatenate([xbf, np.zeros((1, D), BF16)], axis=0)  # pad row

    wfc = np.ascontiguousarray(W_fc, dtype=np.float32).astype(BF16)
    w2 = np.zeros((P, 2 * D), BF16)
    w2[:, 0:D] = np.asarray(W_rate, np.float32).astype(BF16)
    w2[:, D:2 * D] = np.asarray(W_rob, np.float32).astype(BF16)
    brob = np.zeros((1, 2 * D), np.float32)
    brob[0, D:2 * D] = np.asarray(b_rob, np.float32)
    bfcrow = np.asarray(b_fc, np.float32).reshape(1, D).copy()
    onesrow = np.ones((1, D), np.float32)

    cfg.ln_trivial = bool(np.all(np.asarray(ln_gamma) == 1.0)
                          and np.all(np.asarray(ln_beta) == 0.0))
    lnab = np.zeros((P, 2 * D), np.float32)
    lnab[:, :D] = np.asarray(ln_gamma, np.float32)[None, :]
    lnab[:, D:] = np.asarray(ln_beta, np.float32)[None, :]

    core_of = row // NLOC

    # pass 1: per-core per-tile edge counts fix the global S (slot groups/tile)
    percore = []
    S = 1
    for r in range(NC):
        m = core_of == r
        rl = row[m] - r * NLOC
        ce = col[m]
        cnt = np.bincount(rl, minlength=TLP)
        cnt_t = np.bincount(rl // P, minlength=T)
        S = max(S, -(-int(cnt_t.max()) // P))
        percore.append((rl, ce, cnt, cnt_t))
    cfg.S = S
    SP1 = S + 1
    SPT = SP1 * P  # slots per tile incl. self group

    in_maps = []
    for r in range(NC):
        rl, ce, cnt, cnt_t = percore[r]
        # order edges by (tile, col) -> fill each tile's slots densely; the
        # col ordering gives the staging gather some source locality
        order = np.lexsort((ce, rl // P))
        rl_s, ce_s = rl[order], ce[order]
        t_s = rl_s // P
        run_start = np.zeros(T + 1, np.int64)
        np.cumsum(cnt_t, out=run_start[1:])
        pos = np.arange(len(rl_s)) - run_start[t_s]     # slot within tile
        slot = t_s * SPT + pos                           # global slot id

        # staged source rows, SBUF layout [128, T*SP1*128] bf16:
        # partition = slot % 128, free = (slot//128)*128 + d.
        # group S of each tile holds the tile's own 128 rows (self term).
        src = np.full(T * SPT, N, np.int64)              # pads -> zero row
        src[slot] = ce_s
        own = r * NLOC + np.arange(TLP)
        own[NLOC:] = N
        src.reshape(T, SP1, P)[:, S, :] = own.reshape(T, P)
        xe_sb = np.ascontiguousarray(
            xbf_ext[src].reshape(T * SP1, P, D).transpose(1, 0, 2)
        ).reshape(P, T * SP1 * D)

        # rowsr: dst-in-tile per edge slot, -1 for pads -> zero sel column
        rowsr = np.full((P, T * S), -1.0, BF16)
        rowsr[pos % P, t_s * S + pos // P] = (rl_s % P).astype(BF16)

        iotab = np.broadcast_to(
            np.arange(P, dtype=BF16)[None, None, :], (P, S, P)
        ).reshape(P, S * P).copy()

        cnt2 = (2.0 * cnt).astype(np.float32)[None, :]
        diagt = np.zeros((T, P, P), BF16)
        pr = np.arange(P)
        for t in range(T):
            diagt[t, pr, pr] = cnt[t * P:(t + 1) * P].astype(BF16)
        diagt = np.ascontiguousarray(diagt.transpose(1, 0, 2)).reshape(P, T * P)
        degl = np.zeros(TLP, np.float32)
        degl[:NLOC] = degree[r * NLOC:(r + 1) * NLOC].astype(np.float32)
        degf = degl.reshape(T, P).T.copy()

        xTloc = np.zeros((P, TLP), BF16)
        xTloc[:, :NLOC] = xbf[r * NLOC:(r + 1) * NLOC].T

        in_maps.append({
            "xe": xe_sb, "xT": xTloc,
            "wfc": wfc, "w2": w2, "brob": brob, "bfcrow": bfcrow,
            "onesrow": onesrow, "lnab": lnab,
            "iotab": iotab, "rowsr": rowsr,
            "cnt2": cnt2, "diagt": diagt, "degf": degf,
        })
    return in_maps


def build(cfg: Cfg):
    """Build the SPMD Bass program (identical on every core)."""
    NC, T, TLP = cfg.NC, cfg.T, cfg.TLP
    S, SP1, SG, NSG = cfg.S, cfg.SP1, cfg.SG, cfg.NSG
    SPT = SP1 * P
    bf = mybir.dt.bfloat16
    f32 = mybir.dt.float32
    AO = mybir.AluOpType
    AF = mybir.ActivationFunctionType

    nc = bacc.Bacc("TRN2", target_bir_lowering=False, debug=False, num_devices=NC)
    for val in (LN_EPS, 1.0 + EPS):
        cs = nc.alloc_sbuf_tensor(f"const-float32-{val}", [P, 1], f32)
        nc.gpsimd.memset(cs.ap(), val)
        nc.const_aps.aps[(f32, val)] = cs.ap()
    nc.all_engine_barrier()

    d_xe = nc.dram_tensor("xe", [P, T * SPT], bf, kind="ExternalInput").ap()
    d_xT = nc.dram_tensor("xT", [P, TLP], bf, kind="ExternalInput").ap()
    d_wfc = nc.dram_tensor("wfc", [P, D], bf, kind="ExternalInput").ap()
    d_w2 = nc.dram_tensor("w2", [P, 2 * D], bf, kind="ExternalInput").ap()
    d_brob = nc.dram_tensor("brob", [1, 2 * D], f32, kind="ExternalInput").ap()
    d_bfc = nc.dram_tensor("bfcrow", [1, D], f32, kind="ExternalInput").ap()
    d_ones = nc.dram_tensor("onesrow", [1, D], f32, kind="ExternalInput").ap()
    d_lnab = nc.dram_tensor("lnab", [P, 2 * D], f32, kind="ExternalInput").ap()
    d_iota = nc.dram_tensor("iotab", [P, S * P], bf, kind="ExternalInput").ap()
    d_rowsr = nc.dram_tensor("rowsr", [P, T * S], bf, kind="ExternalInput").ap()
    d_cnt2 = nc.dram_tensor("cnt2", [1, TLP], f32, kind="ExternalInput").ap()
    d_diag = nc.dram_tensor("diagt", [P, T * P], bf, kind="ExternalInput").ap()
    d_degf = nc.dram_tensor("degf", [P, T], f32, kind="ExternalInput").ap()
    d_y = nc.dram_tensor("y", [TLP, D], bf, kind="ExternalOutput").ap()

    with tile.TileContext(nc) as tc, ExitStack() as ctx:
        from concourse import library_config
        nc.gpsimd.load_library(library_config.standard)
        consts = ctx.enter_context(tc.tile_pool(name="consts", bufs=1))
        wfc = consts.tile([P, D], bf)
        nc.sync.dma_start(wfc[:], d_wfc[:])
        w2 = consts.tile([P, 2 * D], bf)
        nc.sync.dma_start(w2[:], d_w2[:])
        xlocT = consts.tile([P, TLP], bf)
        nc.sync.dma_start(xlocT[:], d_xT[:])
        brob = consts.tile([1, 2 * D], f32)
        nc.sync.dma_start(brob[:], d_brob[:])
        bfcrow = consts.tile([1, D], f32)
        nc.sync.dma_start(bfcrow[:], d_bfc[:])
        onesr = consts.tile([1, D], f32)
        nc.sync.dma_start(onesr[:], d_ones[:])
        iota = consts.tile([P, S * P], bf)
        nc.sync.dma_start(iota[:], d_iota[:])
        rowsr = consts.tile([P, T * S], bf)
        nc.sync.dma_start(rowsr[:], d_rowsr[:])
        cnt2 = consts.tile([1, TLP], f32)
        nc.sync.dma_start(cnt2[:], d_cnt2[:])
        degf = consts.tile([P, T], f32)
        nc.sync.dma_start(degf[:], d_degf[:])
        lnab = None
        if not cfg.ln_trivial:
            lnab = consts.tile([P, 2 * D], f32)
            nc.sync.dma_start(lnab[:], d_lnab[:])

        xep = ctx.enter_context(tc.tile_pool(name="xep", bufs=2))
        selp = ctx.enter_context(tc.tile_pool(name="selp", bufs=4))
        xw2ps = ctx.enter_context(tc.tile_pool(name="xw2ps", bufs=3, space="PSUM"))
        sxps = ctx.enter_context(tc.tile_pool(name="sxps", bufs=2, space="PSUM"))
        aggps = ctx.enter_context(tc.tile_pool(name="aggps", bufs=2, space="PSUM"))
        sxtp = ctx.enter_context(tc.tile_pool(name="sxtp", bufs=2))
        diagp = ctx.enter_context(tc.tile_pool(name="diagp", bufs=2))
        sgp = ctx.enter_context(tc.tile_pool(name="sgp", bufs=2))
        ysgp = ctx.enter_context(tc.tile_pool(name="ysgp", bufs=1))
        stp = ctx.enter_context(tc.tile_pool(name="stp", bufs=1))

        def emit_compute(sg):
            """Pass 1+2 for super-group sg: GEMMs, sel, x-space segment sums,
            agg.  Returns the context needed by the (deferred) elementwise."""
            t0 = sg * SG
            xe = xep.tile([P, SG * SPT], bf, tag="xe", name="xe")
            nc.sync.dma_start(xe[:], d_xe[:, t0 * SPT:(t0 + SG) * SPT])
            diagsg = diagp.tile([P, SG * P], bf, tag="diag", name="diag")
            nc.sync.dma_start(diagsg[:], d_diag[:, t0 * P:(t0 + SG) * P])

            rate_sg = sgp.tile([P, SG, D], bf, tag="rate", name="rate")
            gam_sg = sgp.tile([P, SG, D], bf, tag="gam", name="gam")
            agg_sg = sgp.tile([P, SG, D], bf, tag="agg", name="agg")

            quads = []       # (sx psum bank, quad width, start tile)
            pending = []     # completed quads awaiting their agg phase

            def flush_quad():
                sxb, qw, q0 = pending.pop(0)
                sxT = sxtp.tile([P, 4, D], bf, tag="sxT", name="sxT")
                nc.scalar.copy(sxT[:, 0:qw, :], sxb[:, 0:qw, :])
                aggb = aggps.tile([P, 4, D], f32, space="PSUM", tag="aggb",
                                  name="aggb")
                for i in range(qw):
                    t = t0 + q0 + i
                    nc.tensor.matmul(out=aggb[:, i, :], lhsT=sxT[:, i, :],
                                     rhs=wfc[:], start=True, stop=False)
                    nc.tensor.matmul(out=aggb[:, i, :],
                                     lhsT=cnt2[0:1, t * P:(t + 1) * P],
                                     rhs=bfcrow[0:1, :], start=False, stop=True)
                nc.scalar.copy(agg_sg[:, q0:q0 + qw, :], aggb[:, 0:qw, :])

            xw2b = None
            sxb = None
            for tl in range(SG):
                t = t0 + tl
                pi = tl % 2
                if pi == 0:
                    pw = min(2, SG - tl)
                    xw2b = xw2ps.tile([P, 2, 2 * D], f32, space="PSUM",
                                      tag="xw2", name="xw2")
                nc.tensor.matmul(out=xw2b[:, pi, :],
                                 lhsT=xlocT[:, t * P:(t + 1) * P],
                                 rhs=w2[:], start=True, stop=False)
                nc.tensor.matmul(out=xw2b[:, pi, :], lhsT=onesr[0:1, :],
                                 rhs=brob[0:1, :], start=False, stop=True)
                sel = selp.tile([P, S * P], bf, tag="sel", name="sel")
                rb = rowsr[:, t * S:(t + 1) * S][:, :, None] \
                    .to_broadcast([P, S, P])
                nc.vector.tensor_tensor(
                    out=sel[:].rearrange("p (s m) -> p s m", s=S),
                    in0=rb, in1=iota.rearrange("p (s m) -> p s m", s=S),
                    op=AO.is_equal)
                qi = tl % 4
                if qi == 0:
                    qw = min(4, SG - tl)
                    sxb = sxps.tile([P, 4, D], f32, space="PSUM", tag="sx",
                                    name="sx")
                    quads.append((sxb, qw, tl))
                for s in range(SP1):
                    g0 = (tl * SP1 + s) * D
                    rhs = (sel[:, s * P:(s + 1) * P] if s < S
                           else diagsg[:, tl * P:(tl + 1) * P])
                    nc.tensor.matmul(out=sxb[:, qi, :], lhsT=xe[:, g0:g0 + D],
                                     rhs=rhs,
                                     start=(s == 0), stop=(s == SP1 - 1))
                if tl == quads[-1][2] + quads[-1][1] - 1:
                    pending.append(quads[-1])
                # delay each quad's agg phase by one tile to hide the
                # PSUM->SBUF copy latency from the PE
                if pending and tl >= pending[0][2] + pending[0][1]:
                    flush_quad()
                if pi + 1 == pw:
                    # rate/gamma for the completed pair
                    b0 = tl - pi
                    nc.scalar.activation(out=rate_sg[:, b0:b0 + pw, :],
                                         in_=xw2b[:, 0:pw, 0:D], func=AF.Exp)
                    nc.scalar.copy(gam_sg[:, b0:b0 + pw, :],
                                   xw2b[:, 0:pw, D:2 * D])
            while pending:
                flush_quad()
            nc.scalar.activation(out=rate_sg[:], in_=rate_sg[:], func=AF.Ln,
                                 bias=1.0)
            return dict(t0=t0, rate=rate_sg, gam=gam_sg, agg=agg_sg)

        def emit_eltwise(cx):
            """Pass 3 for a super-group: batched elementwise + LayerNorm."""
            t0 = cx["t0"]
            rate_sg, gam_sg, agg_sg = cx["rate"], cx["gam"], cx["agg"]
            num_sg = sgp.tile([P, SG, D], bf, tag="num", name="num")
            y0_sg = sgp.tile([P, SG, D], bf, tag="y0", name="y0")
            yt = ysgp.tile([P, SG, D], bf, tag="yt", name="yt")
            st = stp.tile([P, 6 * SG], f32, tag="st", name="st")
            stb = stp.tile([P, 2 * SG], bf, tag="stb", name="stb")
            s1 = st[:, 0 * SG:1 * SG]
            s2 = st[:, 1 * SG:2 * SG]
            mean = st[:, 2 * SG:3 * SG]
            rstd = st[:, 3 * SG:4 * SG]
            msq = st[:, 4 * SG:5 * SG]
            var = st[:, 5 * SG:6 * SG]
            meanb_src = stb[:, 0:SG]
            rstdb_src = stb[:, SG:2 * SG]

            degb = degf[:, t0:t0 + SG][:, :, None].to_broadcast([P, SG, D])
            nc.vector.scalar_tensor_tensor(
                out=num_sg[:], in0=rate_sg[:], scalar=EPS, in1=agg_sg[:],
                op0=AO.add, op1=AO.mult)
            nc.vector.tensor_add(out=num_sg[:], in0=num_sg[:], in1=gam_sg[:])
            # gamma has been consumed: reuse its buffer for the 1/den chain
            invd_sg = gam_sg
            nc.vector.scalar_tensor_tensor(
                out=invd_sg[:], in0=rate_sg[:], scalar=EPS, in1=degb,
                op0=AO.add, op1=AO.mult)
            nc.scalar.activation(out=invd_sg[:], in_=invd_sg[:], func=AF.Ln,
                                 bias=1.0 + EPS)
            nc.scalar.activation(out=invd_sg[:], in_=invd_sg[:], func=AF.Exp,
                                 scale=-1.0)
            nc.vector.tensor_mul(out=y0_sg[:], in0=num_sg[:], in1=invd_sg[:])
            # LayerNorm stats per (node, tile)
            nc.vector.tensor_reduce(out=s1, in_=y0_sg[:],
                                    axis=mybir.AxisListType.X, op=AO.add)
            # reuse num_sg as the square scratch
            nc.vector.tensor_mul(out=num_sg[:], in0=y0_sg[:], in1=y0_sg[:])
            nc.vector.tensor_reduce(out=s2, in_=num_sg[:],
                                    axis=mybir.AxisListType.X, op=AO.add)
            nc.vector.tensor_scalar_mul(out=mean, in0=s1, scalar1=1.0 / D)
            nc.vector.tensor_scalar_mul(out=msq, in0=s2, scalar1=1.0 / D)
            nc.vector.tensor_tensor(out=var, in0=mean, in1=mean, op=AO.mult)
            nc.vector.tensor_sub(out=var, in0=msq, in1=var)
            nc.scalar.activation(out=var, in_=var, func=AF.Ln, bias=LN_EPS)
            nc.scalar.activation(out=rstd, in_=var, func=AF.Exp, scale=-0.5)
            nc.scalar.copy(stb[:], st[:, 2 * SG:4 * SG])
            meanb = meanb_src[:, :, None].to_broadcast([P, SG, D])
            rstdb = rstdb_src[:, :, None].to_broadcast([P, SG, D])
            nc.vector.tensor_sub(out=y0_sg[:], in0=y0_sg[:], in1=meanb)
            if lnab is None:
                nc.vector.tensor_mul(out=yt[:], in0=y0_sg[:], in1=rstdb)
            else:
                nc.vector.tensor_mul(out=y0_sg[:], in0=y0_sg[:], in1=rstdb)
                lg = lnab[:, 0:D][:, None, :].to_broadcast([P, SG, D])
                lb = lnab[:, D:2 * D][:, None, :].to_broadcast([P, SG, D])
                nc.vector.tensor_mul(out=y0_sg[:], in0=y0_sg[:], in1=lg)
                nc.vector.tensor_add(out=yt[:], in0=y0_sg[:], in1=lb)
            dst = d_y[t0 * P:(t0 + SG) * P, :].rearrange("(t p) d -> p t d",
                                                         p=P)
            nc.sync.dma_start(dst, yt[:])

        prev = None
        for sg in range(NSG):
            cx = emit_compute(sg)
            if prev is not None:
                emit_eltwise(prev)
            prev = cx
        emit_eltwise(prev)

    nc.compile()
    return nc


def run(inputs, cfg: Cfg, core_ids=None):
    in_maps = prep(**inputs, cfg=cfg)
    nc = build(cfg)
    res = run_bass_kernel_spmd(nc, in_maps, core_ids=core_ids or list(range(cfg.NC)))
    ys = [res.results[r]["y"][:cfg.NLOC] for r in range(cfg.NC)]
    return np.concatenate(ys, axis=0).astype(np.float32)


def kernel(**inputs):
    cfg = Cfg(N=100_000, E=800_000, NC=8)
    return run(inputs, cfg)
